# revision 23
# baseline (speedup 1.0000x reference)
"""Bahdanau attention on 8 Trainium2 NeuronCores (Bass/Tile).

Data-parallel over batch: B=64 -> 8 rows per core; weights replicated.

Per-core dataflow (BL=8 batch rows, S=2048, H=512):
  keysT  : host-pretransposed keys slice [BL, H, S] (so the U-projection
           matmul can contract over H on the partition dim, streamed fp32r)
  UkT    = U_w @ keys_b^T           PE, fp32r, PSUM [g=128, s=512] tiles
  energy = tanh(UkT + bias_b[g])    ACT, bias = (W_w q_b + W_b + U_b)[g]
  logits = v^T energy               PE, v replicated over all 128 out rows
  softmax over s                    DVE max (negated) + ACT exp w/ accum Z
  context= (exp . keysT) / Z        DVE fused multiply+reduce over s
  attw   = exp / Z                  row 0 of the replicated exp tiles
"""

import sys
from contextlib import ExitStack

import numpy as np

try:
    import concourse.bass as bass  # noqa: F401
except ImportError:  # pragma: no cover
    sys.path.insert(0, "/opt/trn_rl_repo")

import concourse.bacc as bacc
import concourse.tile as tile
from concourse import mybir
from concourse.bass_utils import run_bass_kernel_spmd

B, S, H = 64, 2048, 512
NCORES = 8
BL = B // NCORES  # 8 batch rows per core
ST = 4            # number of s tiles
SW = S // ST      # 512 s per tile
GC = 4            # g (output-H) chunks of 128
HC = 4            # h (contraction) chunks of 128

F32 = mybir.dt.float32
F32R = mybir.dt.float32r

_cache = {}


def build(stage=99, nb=BL, reps=1, loop_reps=0):
    A = mybir.AluOpType
    AF = mybir.ActivationFunctionType
    X = mybir.AxisListType.X

    nc = bacc.Bacc("TRN2", target_bir_lowering=False, debug=False,
                   num_devices=NCORES)

    keysT = nc.dram_tensor("keysT", [BL, H, S], F32R, kind="ExternalInput").ap()
    nCrep = nc.dram_tensor("nC_rep", [128, 1], F32, kind="ExternalInput").ap()
    qT = nc.dram_tensor("qT", [H, BL], F32, kind="ExternalInput").ap()
    WwT = nc.dram_tensor("WwT", [H, H], F32, kind="ExternalInput").ap()
    UwT = nc.dram_tensor("UwT", [H, H], F32R, kind="ExternalInput").ap()
    cb = nc.dram_tensor("cb_rep", [128, GC * BL], F32, kind="ExternalInput").ap()
    vrep = nc.dram_tensor("v_rep", [128, H], F32R, kind="ExternalInput").ap()
    ctx_o = nc.dram_tensor("ctx_o", [BL, H], F32, kind="ExternalOutput").ap()
    attw_o = nc.dram_tensor("attw_o", [BL, S], F32, kind="ExternalOutput").ap()

    with tile.TileContext(nc) as tc, ExitStack() as ctx:
        const = ctx.enter_context(tc.tile_pool(name="const", bufs=1))
        kpool = ctx.enter_context(tc.tile_pool(name="keys", bufs=2 * ST))
        epool = ctx.enter_context(tc.tile_pool(name="energy", bufs=12))
        xpool = ctx.enter_context(tc.tile_pool(name="exp", bufs=2 * ST))
        spool = ctx.enter_context(tc.tile_pool(name="small", bufs=2))
        scr = ctx.enter_context(tc.tile_pool(name="scratch", bufs=2))
        upsum = ctx.enter_context(tc.tile_pool(name="upsum", bufs=3, space="PSUM"))
        lpsum = ctx.enter_context(tc.tile_pool(name="lpsum", bufs=4, space="PSUM"))
        wqps = ctx.enter_context(tc.tile_pool(name="wqps", bufs=1, space="PSUM"))

        # ---- constants / weights
        WwT_sb = const.tile([128, HC * H], F32)
        UwT_sb = const.tile([128, HC * H], F32R)
        qT_sb = const.tile([128, HC * BL], F32)
        for hc in range(HC):
            nc.sync.dma_start(WwT_sb[:, hc * H:(hc + 1) * H],
                              WwT[hc * 128:(hc + 1) * 128, :])
            nc.sync.dma_start(UwT_sb[:, hc * H:(hc + 1) * H],
                              UwT[hc * 128:(hc + 1) * 128, :])
            nc.sync.dma_start(qT_sb[:, hc * BL:(hc + 1) * BL],
                              qT[hc * 128:(hc + 1) * 128, :])
        cb_sb = const.tile([128, GC * BL], F32)
        nc.sync.dma_start(cb_sb[:], cb)
        v_sb = const.tile([128, H], F32R)
        nc.sync.dma_start(v_sb[:], vrep)
        nC_sb = const.tile([128, 1], F32)
        nc.sync.dma_start(nC_sb[:], nCrep)

        # ---- biasT[g, (gc,b)] = (W_w @ q_b + W_b + U_b)[g]   (fp32 matmul)
        biasT = const.tile([128, GC * BL], F32)
        for gc in range(GC):
            wq = wqps.tile([128, BL], F32, tag="wq")
            for hc in range(HC):
                nc.tensor.matmul(
                    wq[:],
                    WwT_sb[:, hc * H + gc * 128: hc * H + (gc + 1) * 128],
                    qT_sb[:, hc * BL:(hc + 1) * BL],
                    start=(hc == 0), stop=(hc == HC - 1))
            nc.vector.tensor_add(biasT[:, gc * BL:(gc + 1) * BL], wq[:],
                                 cb_sb[:, gc * BL:(gc + 1) * BL])

        aw_sb = const.tile([BL, S], F32)
        rz_all = const.tile([BL, 1], F32)

        def _tail(b, st, kt, ens, zp, cparts):
            """dot-v + exp(l - C) + context partials for one s tile."""
            lg = lpsum.tile([128, SW], F32, tag="lg")
            for gc in range(GC):
                nc.tensor.matmul(lg[:], v_sb[:, gc * 128:(gc + 1) * 128],
                                 ens[gc][:],
                                 start=(gc == 0), stop=(gc == GC - 1))
            if stage < 3:
                return
            ex = xpool.tile([128, SW], F32, tag="ex")
            nc.scalar.activation(ex[:], lg[:], AF.Exp,
                                 bias=nC_sb[:, 0:1], scale=1.0,
                                 accum_out=zp[:, st:st + 1])
            if stage >= 4:
                for hc in range(HC):
                    so = scr.tile([128, SW], F32, tag="so")
                    nc.vector.scalar_tensor_tensor(
                        out=so[:],
                        in0=kt[:, hc * SW:(hc + 1) * SW].bitcast(F32),
                        scalar=1.0,
                        in1=ex[:],
                        op0=A.mult, op1=A.mult,
                        accum_out=cparts[:, hc * ST + st: hc * ST + st + 1])
            if stage >= 5:
                nc.sync.dma_start(aw_sb[b:b + 1, st * SW:(st + 1) * SW],
                                  ex[0:1, :])

        def _uk_tile(b, st):
            kt = kpool.tile([128, HC * SW], F32R, tag="kt")
            for hc in range(HC):
                nc.sync.dma_start(
                    kt[:, hc * SW:(hc + 1) * SW],
                    keysT[b, hc * 128:(hc + 1) * 128, st * SW:(st + 1) * SW])
            ens = []
            for gc in range(GC):
                up = upsum.tile([128, SW], F32, tag="up")
                for hc in range(HC):
                    nc.tensor.matmul(
                        up[:],
                        UwT_sb[:, hc * H + gc * 128: hc * H + (gc + 1) * 128],
                        kt[:, hc * SW:(hc + 1) * SW],
                        start=(hc == 0), stop=(hc == HC - 1))
                en = epool.tile([128, SW], F32R, tag="en")
                nc.scalar.activation(en[:], up[:], AF.Tanh,
                                     bias=biasT[:, gc * BL + b: gc * BL + b + 1],
                                     scale=1.0)
                ens.append(en)
            return kt, ens

        def _b_close(b, zp, cparts):
            z = spool.tile([128, 1], F32, tag="z")
            nc.vector.tensor_reduce(z[:], zp[:], axis=X, op=A.add)
            rz = spool.tile([128, 1], F32, tag="rz")
            nc.vector.reciprocal(rz[:], z[:])
            if stage >= 4:
                ctxv = spool.tile([128, HC], F32, tag="ctxv")
                for hc in range(HC):
                    nc.vector.tensor_reduce(ctxv[:, hc:hc + 1],
                                            cparts[:, hc * ST:(hc + 1) * ST],
                                            axis=X, op=A.add)
                ctxs = spool.tile([128, HC], F32, tag="ctxs")
                nc.vector.tensor_scalar_mul(ctxs[:], ctxv[:], rz[:, 0:1])
                for hc in range(HC):
                    nc.sync.dma_start(ctx_o[b, hc * 128:(hc + 1) * 128],
                                      ctxs[:, hc:hc + 1])
            if stage >= 5:
                nc.sync.dma_start(rz_all[b:b + 1, 0:1], rz[0:1, 0:1])

        def _mainbody():
            # streaming softmax: constant shift -C, no per-b barrier;
            # tails run one s-tile behind so PE never waits on ACT
            prev = None
            bstk = [(b, st)
                    for b in [bb for _ in range(reps) for bb in range(nb)]
                    for st in range(ST)]
            zp = cparts = None
            for b, st in bstk:
                if st == 0:
                    zp = spool.tile([128, ST], F32, tag="zp")
                    cparts = spool.tile([128, GC * ST], F32, tag="cparts")
                kt, ens = _uk_tile(b, st)
                if prev is not None and stage >= 2:
                    _tail(*prev)
                    if prev[1] == ST - 1 and stage >= 3:
                        _b_close(prev[0], prev[4], prev[5])
                prev = (b, st, kt, ens, zp, cparts)
            if prev is not None and stage >= 2:
                _tail(*prev)
                if stage >= 3:
                    _b_close(prev[0], prev[4], prev[5])

        if stage >= 1:
            if loop_reps > 0:
                with tc.For_i(0, loop_reps, 1):
                    _mainbody()
            else:
                _mainbody()

        if stage >= 5:
            aw_out = const.tile([BL, S], F32)
            nc.scalar.mul(aw_out[:], aw_sb[:], rz_all[:, 0:1])
            nc.sync.dma_start(attw_o[:, :], aw_out[:])

    nc.compile()
    return nc


def _prep_shared(W_w, W_b, U_w, U_b, v_w):
    WwT = np.ascontiguousarray(W_w.T)
    UwT = np.ascontiguousarray(U_w.T)
    cbv = (U_b + W_b).reshape(GC, 128)  # [gc, p]
    cb = np.ascontiguousarray(
        np.repeat(cbv.T[:, :, None], BL, axis=2).reshape(128, GC * BL))
    vv = v_w.reshape(GC, 128)
    vr = np.ascontiguousarray(
        np.repeat(vv.T[:, :, None], 128, axis=2).reshape(128, H))
    # upper bound on |logits| = |v . tanh(...)| <= sum|v|; exp(l - C) in (0, 1]
    nC = np.full((128, 1), -float(np.abs(v_w).sum()), np.float32)
    return WwT, UwT, cb, vr, nC


def kernel(query, keys, W_w, W_b, U_w, U_b, v_w, v_b):
    query = np.asarray(query, np.float32)
    keys = np.asarray(keys, np.float32)
    W_w = np.asarray(W_w, np.float32)
    W_b = np.asarray(W_b, np.float32)
    U_w = np.asarray(U_w, np.float32)
    U_b = np.asarray(U_b, np.float32)
    v_w = np.asarray(v_w, np.float32)

    if "nc" not in _cache:
        _cache["nc"] = build()
    nc = _cache["nc"]

    WwT, UwT, cb, vr, nC = _prep_shared(W_w, W_b, U_w, U_b, v_w)
    in_maps = []
    for c in range(NCORES):
        sl = slice(c * BL, (c + 1) * BL)
        in_maps.append({
            "keysT": np.ascontiguousarray(keys[sl].transpose(0, 2, 1)),
            "qT": np.ascontiguousarray(query[sl].T),
            "WwT": WwT, "UwT": UwT, "cb_rep": cb, "v_rep": vr, "nC_rep": nC,
        })
    res = run_bass_kernel_spmd(nc, in_maps, core_ids=list(range(NCORES)))
    context = np.concatenate([res.results[c]["ctx_o"] for c in range(NCORES)], 0)
    attw = np.concatenate([res.results[c]["attw_o"] for c in range(NCORES)], 0)
    return context, attw


# revision 30
# speedup vs baseline: 1.2931x; 1.2931x over previous
"""Bahdanau attention on 8 Trainium2 NeuronCores (Bass/Tile).

Data-parallel over batch: B=64 -> 8 rows per core; weights replicated.

Per-core dataflow (BL=8 batch rows, S=2048, H=512):
  keysT  : host-pretransposed keys slice [BL, H, S] (so the U-projection
           matmul can contract over H on the partition dim, streamed fp32r)
  UkT    = U_w @ keys_b^T           PE, fp32r, PSUM [g=128, s=512] tiles
  energy = tanh(UkT + bias_b[g])    ACT, bias = (W_w q_b + W_b + U_b)[g]
  logits = v^T energy               PE, v replicated over all 128 out rows
  softmax over s                    streaming: exp(l - C) with constant
                                    C = sum|v_w| (shift-invariant), ACT exp
                                    with fused accumulate for Z
  context= (exp . keysT) / Z        DVE scalar_tensor_tensor mul+reduce
  attw   = exp / Z                  row 0 of the replicated exp tiles

DMA note: descriptor/instruction count matters a lot on this part —
batched multi-dim APs (one DMA per keysT tile / per-b outputs) measured
~80 us faster than per-chunk DMAs.
"""

import sys
from contextlib import ExitStack

import numpy as np

try:
    import concourse.bass as bass  # noqa: F401
except ImportError:  # pragma: no cover
    sys.path.insert(0, "/opt/trn_rl_repo")

import concourse.bacc as bacc
import concourse.tile as tile
from concourse import mybir
from concourse.bass_utils import run_bass_kernel_spmd

B, S, H = 64, 2048, 512
NCORES = 8
BL = B // NCORES  # 8 batch rows per core
ST = 4            # number of s tiles
SW = S // ST      # 512 s per tile
GC = 4            # g (output-H) chunks of 128
HC = 4            # h (contraction) chunks of 128

F32 = mybir.dt.float32
F32R = mybir.dt.float32r

_cache = {}


def build(stage=99, nb=BL, reps=1, loop_reps=0, v2=True, v3=False, v4=True, v5=True,
          ub=3, lb=4, eb=12, scb=2, kb=None):
    A = mybir.AluOpType
    AF = mybir.ActivationFunctionType
    X = mybir.AxisListType.X

    nc = bacc.Bacc("TRN2", target_bir_lowering=False, debug=False,
                   num_devices=NCORES)

    keysT = nc.dram_tensor("keysT", [BL, H, S], F32R, kind="ExternalInput").ap()
    nCrep = nc.dram_tensor("nC_rep", [128, 1], F32, kind="ExternalInput").ap()
    qT = nc.dram_tensor("qT", [H, BL], F32, kind="ExternalInput").ap()
    WwT = nc.dram_tensor("WwT", [H, H], F32, kind="ExternalInput").ap()
    UwT = nc.dram_tensor("UwT", [H, H], F32R, kind="ExternalInput").ap()
    cb = nc.dram_tensor("cb_rep", [128, GC * BL], F32, kind="ExternalInput").ap()
    vrep = nc.dram_tensor("v_rep", [128, H], F32R, kind="ExternalInput").ap()
    ctx_o = nc.dram_tensor("ctx_o", [BL, H], F32, kind="ExternalOutput").ap()
    attw_o = nc.dram_tensor("attw_o", [BL, S], F32, kind="ExternalOutput").ap()

    with tile.TileContext(nc) as tc, ExitStack() as ctx:
        const = ctx.enter_context(tc.tile_pool(name="const", bufs=1))
        kpool = ctx.enter_context(
            tc.tile_pool(name="keys", bufs=kb or (3 if v3 else 2 * ST)))
        epool = ctx.enter_context(tc.tile_pool(name="energy", bufs=eb))
        xpool = ctx.enter_context(
            tc.tile_pool(name="exp", bufs=2 if (v3 or v4) else 2 * ST))
        spool = ctx.enter_context(tc.tile_pool(name="small", bufs=2))
        scr = ctx.enter_context(tc.tile_pool(name="scratch", bufs=scb))
        upsum = ctx.enter_context(tc.tile_pool(name="upsum", bufs=ub, space="PSUM"))
        lpsum = ctx.enter_context(tc.tile_pool(name="lpsum", bufs=lb, space="PSUM"))
        wqps = ctx.enter_context(tc.tile_pool(name="wqps", bufs=1, space="PSUM"))

        # ---- constants / weights
        WwT_sb = const.tile([128, HC * H], F32)
        UwT_sb = const.tile([128, HC * H], F32R)
        qT_sb = const.tile([128, HC * BL], F32)
        for hc in range(HC):
            nc.sync.dma_start(WwT_sb[:, hc * H:(hc + 1) * H],
                              WwT[hc * 128:(hc + 1) * 128, :])
            nc.sync.dma_start(UwT_sb[:, hc * H:(hc + 1) * H],
                              UwT[hc * 128:(hc + 1) * 128, :])
            nc.sync.dma_start(qT_sb[:, hc * BL:(hc + 1) * BL],
                              qT[hc * 128:(hc + 1) * 128, :])
        cb_sb = const.tile([128, GC * BL], F32)
        nc.sync.dma_start(cb_sb[:], cb)
        v_sb = const.tile([128, H], F32R)
        nc.sync.dma_start(v_sb[:], vrep)
        nC_sb = const.tile([128, 1], F32)
        nc.sync.dma_start(nC_sb[:], nCrep)

        # ---- biasT[g, (gc,b)] = (W_w @ q_b + W_b + U_b)[g]   (fp32 matmul)
        biasT = const.tile([128, GC * BL], F32)
        for gc in range(GC):
            wq = wqps.tile([128, BL], F32, tag="wq")
            for hc in range(HC):
                nc.tensor.matmul(
                    wq[:],
                    WwT_sb[:, hc * H + gc * 128: hc * H + (gc + 1) * 128],
                    qT_sb[:, hc * BL:(hc + 1) * BL],
                    start=(hc == 0), stop=(hc == HC - 1))
            nc.vector.tensor_add(biasT[:, gc * BL:(gc + 1) * BL], wq[:],
                                 cb_sb[:, gc * BL:(gc + 1) * BL])

        aw_sb = const.tile([BL, S], F32)
        rz_all = const.tile([BL, 1], F32)
        rzc = const.tile([128, BL], F32)
        ctx_all = const.tile([128, BL * HC], F32)

        def _kt_slice(kt, st, hc):
            if v3:
                return kt[:, hc * S + st * SW: hc * S + (st + 1) * SW]
            return kt[:, hc * SW:(hc + 1) * SW]

        def _tail(b, st, kt, ens, zp, cparts, exb):
            """dot-v + exp(l - C) + context partials for one s tile."""
            lg = lpsum.tile([128, SW], F32, tag="lg")
            for gc in range(GC):
                nc.tensor.matmul(lg[:], v_sb[:, gc * 128:(gc + 1) * 128],
                                 ens[gc][:],
                                 start=(gc == 0), stop=(gc == GC - 1))
            if stage < 3:
                return
            if v3 or v4:
                ex = exb[:, st * SW:(st + 1) * SW]
            else:
                ex = xpool.tile([128, SW], F32, tag="ex")
            nc.scalar.activation(ex[:], lg[:], AF.Exp,
                                 bias=nC_sb[:, 0:1], scale=1.0,
                                 accum_out=zp[:, st:st + 1])
            if stage >= 4:
                for hc in range(HC):
                    so = scr.tile([128, SW], F32, tag="so")
                    nc.vector.scalar_tensor_tensor(
                        out=so[:],
                        in0=_kt_slice(kt, st, hc).bitcast(F32),
                        scalar=1.0,
                        in1=ex[:],
                        op0=A.mult, op1=A.mult,
                        accum_out=cparts[:, hc * ST + st: hc * ST + st + 1])
            if stage >= 5:
                if v3 or v4:
                    if st == ST - 1:
                        nc.sync.dma_start(aw_sb[b:b + 1, :], exb[0:1, :])
                else:
                    nc.sync.dma_start(aw_sb[b:b + 1, st * SW:(st + 1) * SW],
                                      ex[0:1, :])

        def _load_kt_b(b):
            """v3: all of batch-row b's keysT in one DMA, hc-major layout."""
            kt = kpool.tile([128, HC * S], F32R, tag="kt")
            src = keysT[b].rearrange("(c p) s -> p c s", p=128)
            nc.sync.dma_start(kt[:].rearrange("p (c s) -> p c s", c=HC), src)
            return kt

        def _uk_tile(b, st, kt=None):
            if kt is None:
                kt = kpool.tile([128, HC * SW], F32R, tag="kt")
                if v2:
                    src = keysT[b].rearrange("(c p) s -> p c s", p=128)
                    nc.sync.dma_start(
                        kt[:].rearrange("p (c s) -> p c s", c=HC),
                        src[:, :, st * SW:(st + 1) * SW])
                else:
                    for hc in range(HC):
                        nc.sync.dma_start(
                            kt[:, hc * SW:(hc + 1) * SW],
                            keysT[b, hc * 128:(hc + 1) * 128,
                                  st * SW:(st + 1) * SW])
            ens = []
            for gc in range(GC):
                up = upsum.tile([128, SW], F32, tag="up")
                for hc in range(HC):
                    nc.tensor.matmul(
                        up[:],
                        UwT_sb[:, hc * H + gc * 128: hc * H + (gc + 1) * 128],
                        _kt_slice(kt, st, hc),
                        start=(hc == 0), stop=(hc == HC - 1))
                en = epool.tile([128, SW], F32R, tag="en")
                nc.scalar.activation(en[:], up[:], AF.Tanh,
                                     bias=biasT[:, gc * BL + b: gc * BL + b + 1],
                                     scale=1.0)
                ens.append(en)
            return kt, ens

        def _b_close(b, zp, cparts):
            z = spool.tile([128, 1], F32, tag="z")
            nc.vector.tensor_reduce(z[:], zp[:], axis=X, op=A.add)
            rz = rzc[:, b:b + 1] if v5 else spool.tile([128, 1], F32, tag="rz")
            nc.vector.reciprocal(rz[:], z[:])
            if stage >= 4:
                ctxv = spool.tile([128, HC], F32, tag="ctxv")
                for hc in range(HC):
                    nc.vector.tensor_reduce(ctxv[:, hc:hc + 1],
                                            cparts[:, hc * ST:(hc + 1) * ST],
                                            axis=X, op=A.add)
                ctxs = (ctx_all[:, b * HC:(b + 1) * HC] if v5
                        else spool.tile([128, HC], F32, tag="ctxs"))
                nc.vector.tensor_scalar_mul(ctxs[:], ctxv[:], rz[:, 0:1])
                if v5:
                    pass  # single batched DMA after the b loop
                elif v2:
                    nc.sync.dma_start(
                        ctx_o[b].rearrange("(c p) -> p c", p=128), ctxs[:])
                else:
                    for hc in range(HC):
                        nc.sync.dma_start(ctx_o[b, hc * 128:(hc + 1) * 128],
                                          ctxs[:, hc:hc + 1])
            if stage >= 5 and not v5:
                nc.sync.dma_start(rz_all[b:b + 1, 0:1], rz[0:1, 0:1])

        def _mainbody():
            # streaming softmax: constant shift -C, no per-b barrier;
            # tails run one s-tile behind so PE never waits on ACT
            prev = None
            bstk = [(b, st)
                    for b in [bb for _ in range(reps) for bb in range(nb)]
                    for st in range(ST)]
            zp = cparts = ktb = exb = None
            for b, st in bstk:
                if st == 0:
                    zp = spool.tile([128, ST], F32, tag="zp")
                    cparts = spool.tile([128, GC * ST], F32, tag="cparts")
                    if v3:
                        ktb = _load_kt_b(b)
                    if v3 or v4:
                        exb = xpool.tile([128, S], F32, tag="ex")
                kt, ens = _uk_tile(b, st, kt=ktb)
                if prev is not None and stage >= 2:
                    _tail(*prev)
                    if prev[1] == ST - 1 and stage >= 3:
                        _b_close(prev[0], prev[4], prev[5])
                prev = (b, st, kt, ens, zp, cparts, exb)
            if prev is not None and stage >= 2:
                _tail(*prev)
                if stage >= 3:
                    _b_close(prev[0], prev[4], prev[5])
            if v5:
                if stage >= 4:
                    nc.sync.dma_start(
                        ctx_o[:].rearrange("b (c p) -> p (b c)", p=128),
                        ctx_all[:])
                if stage >= 5:
                    nc.sync.dma_start(rz_all[:, 0:1], rzc[0:1, :])

        if stage >= 1:
            if loop_reps > 0:
                with tc.For_i(0, loop_reps, 1):
                    _mainbody()
            else:
                _mainbody()

        if stage >= 5:
            aw_out = const.tile([BL, S], F32)
            nc.scalar.mul(aw_out[:], aw_sb[:], rz_all[:, 0:1])
            nc.sync.dma_start(attw_o[:, :], aw_out[:])

    nc.compile()
    return nc


def _prep_shared(W_w, W_b, U_w, U_b, v_w):
    WwT = np.ascontiguousarray(W_w.T)
    UwT = np.ascontiguousarray(U_w.T)
    cbv = (U_b + W_b).reshape(GC, 128)  # [gc, p]
    cb = np.ascontiguousarray(
        np.repeat(cbv.T[:, :, None], BL, axis=2).reshape(128, GC * BL))
    vv = v_w.reshape(GC, 128)
    vr = np.ascontiguousarray(
        np.repeat(vv.T[:, :, None], 128, axis=2).reshape(128, H))
    # upper bound on |logits| = |v . tanh(...)| <= sum|v|; exp(l - C) in (0, 1]
    nC = np.full((128, 1), -float(np.abs(v_w).sum()), np.float32)
    return WwT, UwT, cb, vr, nC


def kernel(query, keys, W_w, W_b, U_w, U_b, v_w, v_b):
    query = np.asarray(query, np.float32)
    keys = np.asarray(keys, np.float32)
    W_w = np.asarray(W_w, np.float32)
    W_b = np.asarray(W_b, np.float32)
    U_w = np.asarray(U_w, np.float32)
    U_b = np.asarray(U_b, np.float32)
    v_w = np.asarray(v_w, np.float32)

    if "nc" not in _cache:
        _cache["nc"] = build()
    nc = _cache["nc"]

    WwT, UwT, cb, vr, nC = _prep_shared(W_w, W_b, U_w, U_b, v_w)
    in_maps = []
    for c in range(NCORES):
        sl = slice(c * BL, (c + 1) * BL)
        in_maps.append({
            "keysT": np.ascontiguousarray(keys[sl].transpose(0, 2, 1)),
            "qT": np.ascontiguousarray(query[sl].T),
            "WwT": WwT, "UwT": UwT, "cb_rep": cb, "v_rep": vr, "nC_rep": nC,
        })
    res = run_bass_kernel_spmd(nc, in_maps, core_ids=list(range(NCORES)))
    context = np.concatenate([res.results[c]["ctx_o"] for c in range(NCORES)], 0)
    attw = np.concatenate([res.results[c]["attw_o"] for c in range(NCORES)], 0)
    return context, attw


# revision 31
# speedup vs baseline: 1.3270x; 1.0263x over previous
"""Bahdanau attention on 8 Trainium2 NeuronCores (Bass/Tile).

Data-parallel over batch: B=64 -> 8 rows per core; weights replicated.

Per-core dataflow (BL=8 batch rows, S=2048, H=512):
  keysT  : host-pretransposed keys slice [BL, H, S] (so the U-projection
           matmul can contract over H on the partition dim, streamed fp32r)
  UkT    = U_w @ keys_b^T           PE, fp32r, PSUM [g=128, s=512] tiles
  energy = tanh(UkT + bias_b[g])    ACT, bias = (W_w q_b + W_b + U_b)[g]
  logits = v^T energy               PE, v replicated over all 128 out rows
  softmax over s                    streaming: exp(l - C) with constant
                                    C = sum|v_w| (shift-invariant), ACT exp
                                    with fused accumulate for Z
  context= (exp . keysT) / Z        DVE scalar_tensor_tensor mul+reduce
  attw   = exp / Z                  row 0 of the replicated exp tiles

DMA note: descriptor/instruction count matters a lot on this part —
batched multi-dim APs (one DMA per keysT tile / per-b outputs) measured
~80 us faster than per-chunk DMAs.
"""

import sys
from contextlib import ExitStack

import numpy as np

try:
    import concourse.bass as bass  # noqa: F401
except ImportError:  # pragma: no cover
    sys.path.insert(0, "/opt/trn_rl_repo")

import concourse.bacc as bacc
import concourse.tile as tile
from concourse import mybir
from concourse.bass_utils import run_bass_kernel_spmd

B, S, H = 64, 2048, 512
NCORES = 8
BL = B // NCORES  # 8 batch rows per core
ST = 4            # number of s tiles
SW = S // ST      # 512 s per tile
GC = 4            # g (output-H) chunks of 128
HC = 4            # h (contraction) chunks of 128

F32 = mybir.dt.float32
F32R = mybir.dt.float32r

_cache = {}


def build(stage=99, nb=BL, reps=1, loop_reps=0, v2=True, v3=False, v4=True, v5=True,
          ub=3, lb=4, eb=12, scb=2, kb=None):
    A = mybir.AluOpType
    AF = mybir.ActivationFunctionType
    X = mybir.AxisListType.X

    nc = bacc.Bacc("TRN2", target_bir_lowering=False, debug=False,
                   num_devices=NCORES)

    keysT = nc.dram_tensor("keysT", [BL, H, S], F32R, kind="ExternalInput").ap()
    nCrep = nc.dram_tensor("nC_rep", [128, 1], F32, kind="ExternalInput").ap()
    qT = nc.dram_tensor("qT", [H, BL], F32, kind="ExternalInput").ap()
    WwT = nc.dram_tensor("WwT", [H, H], F32, kind="ExternalInput").ap()
    UwT = nc.dram_tensor("UwT", [H, H], F32R, kind="ExternalInput").ap()
    cb = nc.dram_tensor("cb_rep", [128, GC * BL], F32, kind="ExternalInput").ap()
    vrep = nc.dram_tensor("v_rep", [128, H], F32R, kind="ExternalInput").ap()
    ctx_o = nc.dram_tensor("ctx_o", [BL, H], F32, kind="ExternalOutput").ap()
    attw_o = nc.dram_tensor("attw_o", [BL, S], F32, kind="ExternalOutput").ap()

    with tile.TileContext(nc) as tc, ExitStack() as ctx:
        const = ctx.enter_context(tc.tile_pool(name="const", bufs=1))
        kpool = ctx.enter_context(
            tc.tile_pool(name="keys", bufs=kb or (3 if v3 else 2 * ST)))
        epool = ctx.enter_context(tc.tile_pool(name="energy", bufs=eb))
        xpool = ctx.enter_context(
            tc.tile_pool(name="exp", bufs=2 if (v3 or v4) else 2 * ST))
        spool = ctx.enter_context(tc.tile_pool(name="small", bufs=2))
        scr = ctx.enter_context(tc.tile_pool(name="scratch", bufs=scb))
        upsum = ctx.enter_context(tc.tile_pool(name="upsum", bufs=ub, space="PSUM"))
        lpsum = ctx.enter_context(tc.tile_pool(name="lpsum", bufs=lb, space="PSUM"))
        wqps = ctx.enter_context(tc.tile_pool(name="wqps", bufs=1, space="PSUM"))

        # ---- constants / weights
        WwT_sb = const.tile([128, HC * H], F32)
        UwT_sb = const.tile([128, HC * H], F32R)
        qT_sb = const.tile([128, HC * BL], F32)
        nc.sync.dma_start(WwT_sb[:].rearrange("p (c g) -> p c g", c=HC),
                          WwT.rearrange("(c p) g -> p c g", p=128))
        nc.sync.dma_start(UwT_sb[:].rearrange("p (c g) -> p c g", c=HC),
                          UwT.rearrange("(c p) g -> p c g", p=128))
        nc.sync.dma_start(qT_sb[:].rearrange("p (c b) -> p c b", c=HC),
                          qT.rearrange("(c p) b -> p c b", p=128))
        cb_sb = const.tile([128, GC * BL], F32)
        nc.sync.dma_start(cb_sb[:], cb)
        v_sb = const.tile([128, H], F32R)
        nc.sync.dma_start(v_sb[:], vrep)
        nC_sb = const.tile([128, 1], F32)
        nc.sync.dma_start(nC_sb[:], nCrep)

        # ---- biasT[g, (gc,b)] = (W_w @ q_b + W_b + U_b)[g]   (fp32 matmul)
        biasT = const.tile([128, GC * BL], F32)
        for gc in range(GC):
            wq = wqps.tile([128, BL], F32, tag="wq")
            for hc in range(HC):
                nc.tensor.matmul(
                    wq[:],
                    WwT_sb[:, hc * H + gc * 128: hc * H + (gc + 1) * 128],
                    qT_sb[:, hc * BL:(hc + 1) * BL],
                    start=(hc == 0), stop=(hc == HC - 1))
            nc.vector.tensor_add(biasT[:, gc * BL:(gc + 1) * BL], wq[:],
                                 cb_sb[:, gc * BL:(gc + 1) * BL])

        aw_sb = const.tile([BL, S], F32)
        rz_all = const.tile([BL, 1], F32)
        rzc = const.tile([128, BL], F32)
        ctx_all = const.tile([128, BL * HC], F32)

        def _kt_slice(kt, st, hc):
            if v3:
                return kt[:, hc * S + st * SW: hc * S + (st + 1) * SW]
            return kt[:, hc * SW:(hc + 1) * SW]

        def _tail(b, st, kt, ens, zp, cparts, exb):
            """dot-v + exp(l - C) + context partials for one s tile."""
            lg = lpsum.tile([128, SW], F32, tag="lg")
            for gc in range(GC):
                nc.tensor.matmul(lg[:], v_sb[:, gc * 128:(gc + 1) * 128],
                                 ens[gc][:],
                                 start=(gc == 0), stop=(gc == GC - 1))
            if stage < 3:
                return
            if v3 or v4:
                ex = exb[:, st * SW:(st + 1) * SW]
            else:
                ex = xpool.tile([128, SW], F32, tag="ex")
            nc.scalar.activation(ex[:], lg[:], AF.Exp,
                                 bias=nC_sb[:, 0:1], scale=1.0,
                                 accum_out=zp[:, st:st + 1])
            if stage >= 4:
                for hc in range(HC):
                    so = scr.tile([128, SW], F32, tag="so")
                    nc.vector.scalar_tensor_tensor(
                        out=so[:],
                        in0=_kt_slice(kt, st, hc).bitcast(F32),
                        scalar=1.0,
                        in1=ex[:],
                        op0=A.mult, op1=A.mult,
                        accum_out=cparts[:, hc * ST + st: hc * ST + st + 1])
            if stage >= 5:
                if v3 or v4:
                    if st == ST - 1:
                        nc.sync.dma_start(aw_sb[b:b + 1, :], exb[0:1, :])
                else:
                    nc.sync.dma_start(aw_sb[b:b + 1, st * SW:(st + 1) * SW],
                                      ex[0:1, :])

        def _load_kt_b(b):
            """v3: all of batch-row b's keysT in one DMA, hc-major layout."""
            kt = kpool.tile([128, HC * S], F32R, tag="kt")
            src = keysT[b].rearrange("(c p) s -> p c s", p=128)
            nc.sync.dma_start(kt[:].rearrange("p (c s) -> p c s", c=HC), src)
            return kt

        def _uk_tile(b, st, kt=None):
            if kt is None:
                kt = kpool.tile([128, HC * SW], F32R, tag="kt")
                if v2:
                    src = keysT[b].rearrange("(c p) s -> p c s", p=128)
                    nc.sync.dma_start(
                        kt[:].rearrange("p (c s) -> p c s", c=HC),
                        src[:, :, st * SW:(st + 1) * SW])
                else:
                    for hc in range(HC):
                        nc.sync.dma_start(
                            kt[:, hc * SW:(hc + 1) * SW],
                            keysT[b, hc * 128:(hc + 1) * 128,
                                  st * SW:(st + 1) * SW])
            ens = []
            for gc in range(GC):
                up = upsum.tile([128, SW], F32, tag="up")
                for hc in range(HC):
                    nc.tensor.matmul(
                        up[:],
                        UwT_sb[:, hc * H + gc * 128: hc * H + (gc + 1) * 128],
                        _kt_slice(kt, st, hc),
                        start=(hc == 0), stop=(hc == HC - 1))
                en = epool.tile([128, SW], F32R, tag="en")
                nc.scalar.activation(en[:], up[:], AF.Tanh,
                                     bias=biasT[:, gc * BL + b: gc * BL + b + 1],
                                     scale=1.0)
                ens.append(en)
            return kt, ens

        def _b_close(b, zp, cparts):
            z = spool.tile([128, 1], F32, tag="z")
            nc.vector.tensor_reduce(z[:], zp[:], axis=X, op=A.add)
            rz = rzc[:, b:b + 1] if v5 else spool.tile([128, 1], F32, tag="rz")
            nc.vector.reciprocal(rz[:], z[:])
            if stage >= 4:
                ctxv = spool.tile([128, HC], F32, tag="ctxv")
                for hc in range(HC):
                    nc.vector.tensor_reduce(ctxv[:, hc:hc + 1],
                                            cparts[:, hc * ST:(hc + 1) * ST],
                                            axis=X, op=A.add)
                ctxs = (ctx_all[:, b * HC:(b + 1) * HC] if v5
                        else spool.tile([128, HC], F32, tag="ctxs"))
                nc.vector.tensor_scalar_mul(ctxs[:], ctxv[:], rz[:, 0:1])
                if v5:
                    pass  # single batched DMA after the b loop
                elif v2:
                    nc.sync.dma_start(
                        ctx_o[b].rearrange("(c p) -> p c", p=128), ctxs[:])
                else:
                    for hc in range(HC):
                        nc.sync.dma_start(ctx_o[b, hc * 128:(hc + 1) * 128],
                                          ctxs[:, hc:hc + 1])
            if stage >= 5 and not v5:
                nc.sync.dma_start(rz_all[b:b + 1, 0:1], rz[0:1, 0:1])

        def _mainbody():
            # streaming softmax: constant shift -C, no per-b barrier;
            # tails run one s-tile behind so PE never waits on ACT
            prev = None
            bstk = [(b, st)
                    for b in [bb for _ in range(reps) for bb in range(nb)]
                    for st in range(ST)]
            zp = cparts = ktb = exb = None
            for b, st in bstk:
                if st == 0:
                    zp = spool.tile([128, ST], F32, tag="zp")
                    cparts = spool.tile([128, GC * ST], F32, tag="cparts")
                    if v3:
                        ktb = _load_kt_b(b)
                    if v3 or v4:
                        exb = xpool.tile([128, S], F32, tag="ex")
                kt, ens = _uk_tile(b, st, kt=ktb)
                if prev is not None and stage >= 2:
                    _tail(*prev)
                    if prev[1] == ST - 1 and stage >= 3:
                        _b_close(prev[0], prev[4], prev[5])
                prev = (b, st, kt, ens, zp, cparts, exb)
            if prev is not None and stage >= 2:
                _tail(*prev)
                if stage >= 3:
                    _b_close(prev[0], prev[4], prev[5])
            if v5:
                if stage >= 4:
                    nc.sync.dma_start(
                        ctx_o[:].rearrange("b (c p) -> p (b c)", p=128),
                        ctx_all[:])
                if stage >= 5:
                    nc.sync.dma_start(rz_all[:, 0:1], rzc[0:1, :])

        if stage >= 1:
            if loop_reps > 0:
                with tc.For_i(0, loop_reps, 1):
                    _mainbody()
            else:
                _mainbody()

        if stage >= 5:
            aw_out = const.tile([BL, S], F32)
            nc.scalar.mul(aw_out[:], aw_sb[:], rz_all[:, 0:1])
            nc.sync.dma_start(attw_o[:, :], aw_out[:])

    nc.compile()
    return nc


def _prep_shared(W_w, W_b, U_w, U_b, v_w):
    WwT = np.ascontiguousarray(W_w.T)
    UwT = np.ascontiguousarray(U_w.T)
    cbv = (U_b + W_b).reshape(GC, 128)  # [gc, p]
    cb = np.ascontiguousarray(
        np.repeat(cbv.T[:, :, None], BL, axis=2).reshape(128, GC * BL))
    vv = v_w.reshape(GC, 128)
    vr = np.ascontiguousarray(
        np.repeat(vv.T[:, :, None], 128, axis=2).reshape(128, H))
    # upper bound on |logits| = |v . tanh(...)| <= sum|v|; exp(l - C) in (0, 1]
    nC = np.full((128, 1), -float(np.abs(v_w).sum()), np.float32)
    return WwT, UwT, cb, vr, nC


def kernel(query, keys, W_w, W_b, U_w, U_b, v_w, v_b):
    query = np.asarray(query, np.float32)
    keys = np.asarray(keys, np.float32)
    W_w = np.asarray(W_w, np.float32)
    W_b = np.asarray(W_b, np.float32)
    U_w = np.asarray(U_w, np.float32)
    U_b = np.asarray(U_b, np.float32)
    v_w = np.asarray(v_w, np.float32)

    if "nc" not in _cache:
        _cache["nc"] = build()
    nc = _cache["nc"]

    WwT, UwT, cb, vr, nC = _prep_shared(W_w, W_b, U_w, U_b, v_w)
    in_maps = []
    for c in range(NCORES):
        sl = slice(c * BL, (c + 1) * BL)
        in_maps.append({
            "keysT": np.ascontiguousarray(keys[sl].transpose(0, 2, 1)),
            "qT": np.ascontiguousarray(query[sl].T),
            "WwT": WwT, "UwT": UwT, "cb_rep": cb, "v_rep": vr, "nC_rep": nC,
        })
    res = run_bass_kernel_spmd(nc, in_maps, core_ids=list(range(NCORES)))
    context = np.concatenate([res.results[c]["ctx_o"] for c in range(NCORES)], 0)
    attw = np.concatenate([res.results[c]["attw_o"] for c in range(NCORES)], 0)
    return context, attw
